# revision 12
# baseline (speedup 1.0000x reference)
"""NT-Xent contrastive loss on 8 TRN2 NeuronCores — v4 (triangle + host-reduce).

Host ships q = fp8(16 * z/|z|); the device does ONLY similarity matmuls
and the exp, shipping the exp tiles straight back to DRAM.  All
reductions (row sums, symmetry column sums, positive-pair diagonals)
happen on the host in float64.

Per core (own block = 1024 rows of z = concat(z_i, z_j), rotated so rot
block g = abs block (c+g)%8):

  - zq [512, 5120] fp8: rotated q^T, groups 0..4; group 0 is the own
    block and doubles as the matmul rhs (loaded as two i-halves so the
    first matmuls start after 256 KB).
  - rhs4 [512, 1024] fp8: own block with i-halves swapped for cores>=4
    (data-driven g4 quadrant split — the program stays SPMD-uniform
    while core pairs (c, c+4) compute complementary quadrants of their
    shared pair block).
  - g0 (the diagonal block) computes only the upper triangle at 128-row
    granularity: 3 ragged units instead of 4 full ones; the host
    reconstructs the lower triangle from column sums.
  - total exp work: 4.0625 block-equivalents per core (theoretical
    minimum for this 8-way symmetric decomposition), as 17 psum units
    built from chained fp8-DoubleRow matmuls; one wide ScalarE exp per
    unit (constant scale 2/256) — ScalarE is the bottleneck engine.
  - a few dummy matmuls at kernel start warm the PE out of its cold
    p-state while the input DMAs stream.
  - exp tiles ship as fp8e4 (g0..3; the reductions average out the
    quantization noise) and bf16 (g4, which carries the positive-pair
    diagonals for cores 0-3).

Host: den[r] = rotated row sums + column sums of the shipped tiles
minus exp(2) self-sim; pos from cores 0-3; loss = mean(ln den - ln pos).
"""

import os
import sys

for _p in ("/opt/trn_rl_repo", "/opt/pypackages"):
    if os.path.isdir(_p) and _p not in sys.path:
        sys.path.append(_p)

import numpy as np

B = 4096
D = 512
N2 = 2 * B                  # 8192 rows total
NCORES = 8
RPC = N2 // NCORES          # 1024 rows per core
GW = 1024
NG = 5                      # lhsT groups (g4 computed as half-units)
ZSCALE = 16.0               # fp8 scale for q = 16 * u
TAU_INV = 2.0               # 1 / temperature
SC_MUL = TAU_INV / (ZSCALE * ZSCALE)   # exp scale on raw psum
TILE = 128 * 2048

# g0 upper-triangle units: lists of (m, i_start, width)
G0_UNITS = [
    [(0, 0, 1024), (1, 128, 896)],
    [(2, 256, 768), (3, 384, 640)],
    [(4, 512, 512), (5, 640, 384), (6, 768, 256), (7, 896, 128)],
]
# fp8 output slots: 0..2 g0 ragged units, 3..14 g1..3 pairs (mp 0..3)
UNITS8 = [(i, 0, segs) for i, segs in enumerate(G0_UNITS)] + [
    (3 + (g - 1) * 4 + mp, g, [(2 * mp, 0, 1024), (2 * mp + 1, 0, 1024)])
    for g in (1, 2, 3) for mp in range(4)]

_NC_CACHE = {}


def _build_nc():
    from contextlib import ExitStack

    import concourse.bacc as bacc
    import concourse.mybir as mybir
    import concourse.tile as tile

    f32 = mybir.dt.float32
    bf16 = mybir.dt.bfloat16
    f8 = mybir.dt.float8e4
    AF = mybir.ActivationFunctionType
    DR = mybir.MatmulPerfMode.DoubleRow

    nc = bacc.Bacc("TRN2", target_bir_lowering=False, debug=False,
                   num_devices=NCORES)
    zq_dram = nc.dram_tensor("zq", [D, NG * GW], f8,
                             kind="ExternalInput").ap()
    rhs4_dram = nc.dram_tensor("rhs4", [D, GW], f8,
                               kind="ExternalInput").ap()
    oej8_dram = nc.dram_tensor("oej8", [15 * TILE], f8,
                               kind="ExternalOutput").ap()
    oej16_dram = nc.dram_tensor("oej16", [2 * TILE], bf16,
                                kind="ExternalOutput").ap()

    with ExitStack() as ctx:
        tc = ctx.enter_context(tile.TileContext(nc))
        const = ctx.enter_context(tc.tile_pool(name="const", bufs=1))
        keep = ctx.enter_context(tc.tile_pool(name="keep", bufs=1))
        pej = ctx.enter_context(tc.tile_pool(name="pej", bufs=4))
        pps = ctx.enter_context(tc.tile_pool(name="pps", bufs=2,
                                             space="PSUM"))

        dum = const.tile([1, 1], f32, name="dum", tag="dum")
        dumo = const.tile([1, 1], bf16, name="dumo", tag="dumo")

        # zq0 as two i-halves so the first matmuls start earlier
        zq0h = [keep.tile([128, 4, 512], f8, name=f"zq0_{h}", tag=f"zq0_{h}")
                for h in range(2)]
        zqt = {g: keep.tile([128, 4, GW], f8, name=f"zqt_{g}",
                            tag=f"zqt_{g}") for g in (1, 2, 3, 4)}
        rhs4t = keep.tile([128, 4, GW], f8, name="rhs4t", tag="rhs4t")
        ej4 = [keep.tile([128, 2048], bf16, name=f"ej4_{u}", tag=f"ej4_{u}")
               for u in range(2)]

        # Preload the Exp activation table while input DMAs stream.
        nc.vector.memset(dum[:], 0.0)
        nc.scalar.activation(out=dumo[:], in_=dum[:], func=AF.Exp)

        for h in range(2):
            nc.sync.dma_start(
                out=zq0h[h][:],
                in_=zq_dram[:, h * 512:(h + 1) * 512]
                .rearrange("(j p) n -> p j n", p=128))
        nc.sync.dma_start(
            out=zqt[1][:],
            in_=zq_dram[:, GW:2 * GW].rearrange("(j p) n -> p j n", p=128))
        nc.sync.dma_start(
            out=zqt[4][:],
            in_=zq_dram[:, 4 * GW:5 * GW].rearrange("(j p) n -> p j n",
                                                    p=128))
        nc.sync.dma_start(
            out=rhs4t[:],
            in_=rhs4_dram.rearrange("(j p) n -> p j n", p=128))
        for g in (2, 3):
            nc.sync.dma_start(
                out=zqt[g][:],
                in_=zq_dram[:, g * GW:(g + 1) * GW]
                .rearrange("(j p) n -> p j n", p=128))

        def unit8(slot, g, segs, h_first=False):
            """Ragged unit: segs = [(m, i0, w)...]; exp -> fp8 slot."""
            W = sum(w for _, _, w in segs)
            ps = pps.tile([128, 2048], f32, name=f"ps_{slot}", tag="ps")
            chunks = []
            off = 0
            for (m, i0, w) in segs:
                o, i = off, i0
                while w > 0:
                    cw = min(512 - i % 512, w)
                    chunks.append((o, m, i, cw))
                    o += cw
                    i += cw
                    w -= cw
                off = o
            if h_first:               # first unit: zq0 half-0 chunks first
                chunks.sort(key=lambda ch: ch[2] // 512)
            for (o, m, i, cw) in chunks:
                h, ir = i // 512, i % 512
                for kp in range(2):
                    if g == 0:
                        lhsT = zq0h[m // 4][:, 2 * kp:2 * kp + 2,
                                            (m % 4) * 128:(m % 4 + 1) * 128]
                    else:
                        lhsT = zqt[g][:, 2 * kp:2 * kp + 2,
                                      m * 128:(m + 1) * 128]
                    nc.tensor.matmul(
                        ps[:, o:o + cw],
                        lhsT=lhsT,
                        rhs=zq0h[h][:, 2 * kp:2 * kp + 2, ir:ir + cw],
                        start=(kp == 0), stop=(kp == 1), perf_mode=DR)
            ej = pej.tile([128, 2048], f8, name=f"ej_{slot}", tag="ej")
            nc.scalar.activation(out=ej[:, 0:W], in_=ps[:, 0:W],
                                 func=AF.Exp, scale=SC_MUL)
            nc.gpsimd.dma_start(
                out=oej8_dram[slot * TILE:slot * TILE + 128 * W]
                .rearrange("(p n) -> p n", p=128), in_=ej[:, 0:W])

        def unit_g4(u):
            """[128, 4, 512] psum: m-blocks (4u..4u+3) x own i-half."""
            ps = pps.tile([128, 2048], f32, name=f"ps4_{u}", tag="ps")
            for seg in range(4):
                m = u * 4 + seg
                for kp in range(2):
                    nc.tensor.matmul(
                        ps[:, seg * 512:(seg + 1) * 512],
                        lhsT=zqt[4][:, 2 * kp:2 * kp + 2,
                                    m * 128:(m + 1) * 128],
                        rhs=rhs4t[:, 2 * kp:2 * kp + 2,
                                  u * 512:(u + 1) * 512],
                        start=(kp == 0), stop=(kp == 1), perf_mode=DR)
            nc.scalar.activation(out=ej4[u][:], in_=ps[:], func=AF.Exp,
                                 scale=SC_MUL)
            nc.gpsimd.dma_start(
                out=oej16_dram[u * TILE:(u + 1) * TILE]
                .rearrange("(p n) -> p n", p=128), in_=ej4[u][:])

        # schedule follows DMA arrival order; small g0 units last
        unit8(0, 0, G0_UNITS[0], h_first=True)
        for mp in range(4):
            unit8(3 + mp, 1, [(2 * mp, 0, 1024), (2 * mp + 1, 0, 1024)])
        unit_g4(0)
        unit_g4(1)
        for g in (2, 3):
            for mp in range(4):
                slot = 3 + (g - 1) * 4 + mp
                unit8(slot, g, [(2 * mp, 0, 1024), (2 * mp + 1, 0, 1024)])
        unit8(1, 0, G0_UNITS[1])
        unit8(2, 0, G0_UNITS[2])

    nc.compile()
    return nc


def _get_nc():
    if "nc" not in _NC_CACHE:
        _NC_CACHE["nc"] = _build_nc()
    return _NC_CACHE["nc"]


def _in_maps(z):
    import ml_dtypes
    u = z / np.linalg.norm(z, axis=1, keepdims=True)
    q = (ZSCALE * u).astype(ml_dtypes.float8_e4m3)        # [N2, D]
    qT = np.ascontiguousarray(q.T)                        # [D, N2]
    qT2 = np.concatenate([qT, qT[:, :NG * GW]], axis=1)
    maps = []
    for c in range(NCORES):
        zq = np.ascontiguousarray(qT2[:, RPC * c:RPC * c + NG * GW])
        own = qT[:, RPC * c:RPC * (c + 1)]
        if c < 4:
            rhs4 = np.ascontiguousarray(own)
        else:
            rhs4 = np.ascontiguousarray(
                np.concatenate([own[:, 512:], own[:, :512]], axis=1))
        maps.append({"zq": zq, "rhs4": rhs4})
    return maps


def _post(oej8s, oej16s):
    """Host reduction of the shipped exp tiles into the scalar loss."""
    den = np.zeros(N2, np.float64)
    pos = np.zeros(N2, np.float64)
    p128 = np.arange(128)
    for c in range(NCORES):
        t8 = np.asarray(oej8s[c], np.float32)
        iabs = RPC * c + np.arange(GW)
        for (slot, g, segs) in UNITS8:
            W = sum(w for _, _, w in segs)
            arr = t8[slot * TILE:slot * TILE + 128 * W].reshape(128, W)
            off = 0
            for (m, i0, w) in segs:
                T = arr[:, off:off + w]
                off += w
                jabs = (RPC * c + GW * g + m * 128 + p128) % N2
                den[jabs] += T.sum(axis=1, dtype=np.float64)
                if g == 0:
                    den[RPC * c + i0 + 128 + np.arange(w - 128)] += \
                        T[:, 128:].sum(axis=0, dtype=np.float64)
                else:
                    den[RPC * c + i0 + np.arange(w)] += \
                        T.sum(axis=0, dtype=np.float64)
        t16 = np.asarray(oej16s[c], np.float32).reshape(2, 128, 4, 512)
        for uu in range(2):
            T = t16[uu]                            # [128 p, 4 seg, 512 iw]
            rows = T.sum(axis=2, dtype=np.float64)       # [p, seg]
            jabs = (RPC * c + 4096 + uu * 512 + np.arange(512)) % N2
            den[jabs] += rows.T.reshape(512)       # j = seg*128 + p
            cols = T.sum(axis=(0, 1), dtype=np.float64)  # [512]
            off = uu * 512 if c < 4 else (1 - uu) * 512
            den[RPC * c + off + np.arange(512)] += cols
            if c < 4:
                for seg in range(4):
                    x = RPC * c + uu * 512 + seg * 128 + p128
                    d = T[p128, seg, seg * 128 + p128]
                    pos[x] = d
                    pos[x + 4096] = d
    den -= np.exp(TAU_INV)
    rows = np.log(den) - np.log(pos)
    return np.float32(np.mean(rows))


def kernel(z_i: np.ndarray, z_j: np.ndarray) -> np.ndarray:
    from concourse.bass_interp import get_hw_module
    from concourse.bass_utils import run_bass_kernel_spmd

    z = np.concatenate([np.asarray(z_i, np.float32),
                        np.asarray(z_j, np.float32)], axis=0)
    nc = _get_nc()
    old_m = nc.m
    nc.m = get_hw_module(nc.m)
    try:
        res = run_bass_kernel_spmd(nc, _in_maps(z),
                                   core_ids=list(range(NCORES)))
    finally:
        nc.m = old_m

    return _post([res.results[c]["oej8"] for c in range(NCORES)],
                 [res.results[c]["oej16"] for c in range(NCORES)])


# revision 18
# speedup vs baseline: 1.0259x; 1.0259x over previous
"""NT-Xent contrastive loss on 8 TRN2 NeuronCores — v4 (triangle + host-reduce).

Host ships q = fp8(16 * z/|z|); the device does ONLY similarity matmuls
and the exp, shipping the exp tiles straight back to DRAM.  All
reductions (row sums, symmetry column sums, positive-pair diagonals)
happen on the host in float64.

Per core (own block = 1024 rows of z = concat(z_i, z_j), rotated so rot
block g = abs block (c+g)%8):

  - zq [512, 5120] fp8: rotated q^T, groups 0..4; group 0 is the own
    block and doubles as the matmul rhs (loaded as two i-halves so the
    first matmuls start after 256 KB).
  - rhs4 [512, 1024] fp8: own block with i-halves swapped for cores>=4
    (data-driven g4 quadrant split — the program stays SPMD-uniform
    while core pairs (c, c+4) compute complementary quadrants of their
    shared pair block).
  - g0 (the diagonal block) computes only the upper triangle at 128-row
    granularity: 3 ragged units instead of 4 full ones; the host
    reconstructs the lower triangle from column sums.
  - total exp work: 4.0625 block-equivalents per core (theoretical
    minimum for this 8-way symmetric decomposition), as 17 psum units
    built from chained fp8-DoubleRow matmuls; one wide ScalarE exp per
    unit (constant scale 2/256) — ScalarE is the bottleneck engine.
  - a few dummy matmuls at kernel start warm the PE out of its cold
    p-state while the input DMAs stream.
  - exp tiles ship as fp8e4 (g0..3; the reductions average out the
    quantization noise) and bf16 (g4, which carries the positive-pair
    diagonals for cores 0-3).

Host: den[r] = rotated row sums + column sums of the shipped tiles
minus exp(2) self-sim; pos from cores 0-3; loss = mean(ln den - ln pos).
"""

import os
import sys

for _p in ("/opt/trn_rl_repo", "/opt/pypackages"):
    if os.path.isdir(_p) and _p not in sys.path:
        sys.path.append(_p)

import numpy as np

B = 4096
D = 512
N2 = 2 * B                  # 8192 rows total
NCORES = 8
RPC = N2 // NCORES          # 1024 rows per core
GW = 1024
NG = 5                      # lhsT groups (g4 computed as half-units)
ZSCALE = 16.0               # fp8 scale for q = 16 * u
TAU_INV = 2.0               # 1 / temperature
SC_MUL = TAU_INV / (ZSCALE * ZSCALE)   # exp scale on raw psum
TILE = 128 * 2048

# g0 upper-triangle units: lists of (m, i_start, width)
G0_UNITS = [
    [(0, 0, 1024), (1, 128, 896)],
    [(2, 256, 768), (3, 384, 640)],
    [(4, 512, 512), (5, 640, 384), (6, 768, 256), (7, 896, 128)],
]
# fp8 output slots: 0..2 g0 ragged units, 3..14 g1..3 pairs (mp 0..3)
UNITS8 = [(i, 0, segs) for i, segs in enumerate(G0_UNITS)] + [
    (3 + (g - 1) * 4 + mp, g, [(2 * mp, 0, 1024), (2 * mp + 1, 0, 1024)])
    for g in (1, 2, 3) for mp in range(4)]

_NC_CACHE = {}


def _build_nc():
    from contextlib import ExitStack

    import concourse.bacc as bacc
    import concourse.mybir as mybir
    import concourse.tile as tile

    f32 = mybir.dt.float32
    bf16 = mybir.dt.bfloat16
    f8 = mybir.dt.float8e4
    AF = mybir.ActivationFunctionType
    DR = mybir.MatmulPerfMode.DoubleRow

    nc = bacc.Bacc("TRN2", target_bir_lowering=False, debug=False,
                   num_devices=NCORES)
    zq_dram = nc.dram_tensor("zq", [D, NG * GW], f8,
                             kind="ExternalInput").ap()
    rhs4_dram = nc.dram_tensor("rhs4", [D, GW], f8,
                               kind="ExternalInput").ap()
    oej8_dram = nc.dram_tensor("oej8", [15 * TILE], f8,
                               kind="ExternalOutput").ap()
    oej16_dram = nc.dram_tensor("oej16", [2 * TILE], bf16,
                                kind="ExternalOutput").ap()

    with ExitStack() as ctx:
        tc = ctx.enter_context(tile.TileContext(nc))
        const = ctx.enter_context(tc.tile_pool(name="const", bufs=1))
        keep = ctx.enter_context(tc.tile_pool(name="keep", bufs=1))
        pej = ctx.enter_context(tc.tile_pool(name="pej", bufs=4))
        pps = ctx.enter_context(tc.tile_pool(name="pps", bufs=2,
                                             space="PSUM"))

        dum = const.tile([1, 1], f32, name="dum", tag="dum")
        dumo = const.tile([1, 1], bf16, name="dumo", tag="dumo")

        zqt = {g: keep.tile([128, 4, GW], f8, name=f"zqt_{g}",
                            tag=f"zqt_{g}") for g in (0, 1, 2, 3, 4)}
        rhs4t = keep.tile([128, 4, GW], f8, name="rhs4t", tag="rhs4t")
        ej4 = [keep.tile([128, 2048], bf16, name=f"ej4_{u}", tag=f"ej4_{u}")
               for u in range(2)]

        # Preload the Exp activation table while input DMAs stream.
        nc.vector.memset(dum[:], 0.0)
        nc.scalar.activation(out=dumo[:], in_=dum[:], func=AF.Exp)

        nc.sync.dma_start(
            out=zqt[0][:],
            in_=zq_dram[:, 0:GW].rearrange("(j p) n -> p j n", p=128))
        nc.sync.dma_start(
            out=zqt[1][:],
            in_=zq_dram[:, GW:2 * GW].rearrange("(j p) n -> p j n", p=128))
        nc.sync.dma_start(
            out=zqt[4][:],
            in_=zq_dram[:, 4 * GW:5 * GW].rearrange("(j p) n -> p j n",
                                                    p=128))
        nc.sync.dma_start(
            out=rhs4t[:],
            in_=rhs4_dram.rearrange("(j p) n -> p j n", p=128))
        for g in (2, 3):
            nc.sync.dma_start(
                out=zqt[g][:],
                in_=zq_dram[:, g * GW:(g + 1) * GW]
                .rearrange("(j p) n -> p j n", p=128))

        def unit8(slot, g, segs):
            """Ragged unit: segs = [(m, i0, w)...]; exp -> fp8 slot."""
            W = sum(w for _, _, w in segs)
            ps = pps.tile([128, 2048], f32, name=f"ps_{slot}", tag="ps")
            chunks = []
            off = 0
            for (m, i0, w) in segs:
                o, i = off, i0
                while w > 0:
                    cw = min(512 - i % 512, w)
                    chunks.append((o, m, i, cw))
                    o += cw
                    i += cw
                    w -= cw
                off = o
            for (o, m, i, cw) in chunks:
                for kp in range(2):
                    nc.tensor.matmul(
                        ps[:, o:o + cw],
                        lhsT=zqt[g][:, 2 * kp:2 * kp + 2,
                                    m * 128:(m + 1) * 128],
                        rhs=zqt[0][:, 2 * kp:2 * kp + 2, i:i + cw],
                        start=(kp == 0), stop=(kp == 1), perf_mode=DR)
            ej = pej.tile([128, 2048], f8, name=f"ej_{slot}", tag="ej")
            nc.scalar.activation(out=ej[:, 0:W], in_=ps[:, 0:W],
                                 func=AF.Exp, scale=SC_MUL)
            nc.gpsimd.dma_start(
                out=oej8_dram[slot * TILE:slot * TILE + 128 * W]
                .rearrange("(p n) -> p n", p=128), in_=ej[:, 0:W])

        def unit_g4(u):
            """[128, 4, 512] psum: m-blocks (4u..4u+3) x own i-half."""
            ps = pps.tile([128, 2048], f32, name=f"ps4_{u}", tag="ps")
            for seg in range(4):
                m = u * 4 + seg
                for kp in range(2):
                    nc.tensor.matmul(
                        ps[:, seg * 512:(seg + 1) * 512],
                        lhsT=zqt[4][:, 2 * kp:2 * kp + 2,
                                    m * 128:(m + 1) * 128],
                        rhs=rhs4t[:, 2 * kp:2 * kp + 2,
                                  u * 512:(u + 1) * 512],
                        start=(kp == 0), stop=(kp == 1), perf_mode=DR)
            nc.scalar.activation(out=ej4[u][:], in_=ps[:], func=AF.Exp,
                                 scale=SC_MUL)
            nc.gpsimd.dma_start(
                out=oej16_dram[u * TILE:(u + 1) * TILE]
                .rearrange("(p n) -> p n", p=128), in_=ej4[u][:])

        # schedule follows DMA arrival order: the two big g0 units first
        # (they need only zq0, and warm the PE while zq1.. stream in)
        unit8(0, 0, G0_UNITS[0])
        unit8(1, 0, G0_UNITS[1])
        for mp in range(4):
            unit8(3 + mp, 1, [(2 * mp, 0, 1024), (2 * mp + 1, 0, 1024)])
        unit_g4(0)
        unit_g4(1)
        for g in (2, 3):
            for mp in range(4):
                slot = 3 + (g - 1) * 4 + mp
                unit8(slot, g, [(2 * mp, 0, 1024), (2 * mp + 1, 0, 1024)])
        unit8(2, 0, G0_UNITS[2])

    nc.compile()
    return nc


def _get_nc():
    if "nc" not in _NC_CACHE:
        _NC_CACHE["nc"] = _build_nc()
    return _NC_CACHE["nc"]


def _in_maps(z):
    import ml_dtypes
    u = z / np.linalg.norm(z, axis=1, keepdims=True)
    q = (ZSCALE * u).astype(ml_dtypes.float8_e4m3)        # [N2, D]
    qT = np.ascontiguousarray(q.T)                        # [D, N2]
    qT2 = np.concatenate([qT, qT[:, :NG * GW]], axis=1)
    maps = []
    for c in range(NCORES):
        zq = np.ascontiguousarray(qT2[:, RPC * c:RPC * c + NG * GW])
        own = qT[:, RPC * c:RPC * (c + 1)]
        if c < 4:
            rhs4 = np.ascontiguousarray(own)
        else:
            rhs4 = np.ascontiguousarray(
                np.concatenate([own[:, 512:], own[:, :512]], axis=1))
        maps.append({"zq": zq, "rhs4": rhs4})
    return maps


def _post(oej8s, oej16s):
    """Host reduction of the shipped exp tiles into the scalar loss."""
    den = np.zeros(N2, np.float64)
    pos = np.zeros(N2, np.float64)
    p128 = np.arange(128)
    for c in range(NCORES):
        t8 = np.asarray(oej8s[c], np.float32)
        iabs = RPC * c + np.arange(GW)
        for (slot, g, segs) in UNITS8:
            W = sum(w for _, _, w in segs)
            arr = t8[slot * TILE:slot * TILE + 128 * W].reshape(128, W)
            off = 0
            for (m, i0, w) in segs:
                T = arr[:, off:off + w]
                off += w
                jabs = (RPC * c + GW * g + m * 128 + p128) % N2
                den[jabs] += T.sum(axis=1, dtype=np.float64)
                if g == 0:
                    den[RPC * c + i0 + 128 + np.arange(w - 128)] += \
                        T[:, 128:].sum(axis=0, dtype=np.float64)
                else:
                    den[RPC * c + i0 + np.arange(w)] += \
                        T.sum(axis=0, dtype=np.float64)
        t16 = np.asarray(oej16s[c], np.float32).reshape(2, 128, 4, 512)
        for uu in range(2):
            T = t16[uu]                            # [128 p, 4 seg, 512 iw]
            rows = T.sum(axis=2, dtype=np.float64)       # [p, seg]
            jabs = (RPC * c + 4096 + uu * 512 + np.arange(512)) % N2
            den[jabs] += rows.T.reshape(512)       # j = seg*128 + p
            cols = T.sum(axis=(0, 1), dtype=np.float64)  # [512]
            off = uu * 512 if c < 4 else (1 - uu) * 512
            den[RPC * c + off + np.arange(512)] += cols
            if c < 4:
                for seg in range(4):
                    x = RPC * c + uu * 512 + seg * 128 + p128
                    d = T[p128, seg, seg * 128 + p128]
                    pos[x] = d
                    pos[x + 4096] = d
    den -= np.exp(TAU_INV)
    rows = np.log(den) - np.log(pos)
    return np.float32(np.mean(rows))


def kernel(z_i: np.ndarray, z_j: np.ndarray) -> np.ndarray:
    from concourse.bass_interp import get_hw_module
    from concourse.bass_utils import run_bass_kernel_spmd

    z = np.concatenate([np.asarray(z_i, np.float32),
                        np.asarray(z_j, np.float32)], axis=0)
    in_maps = _in_maps(z)
    last_err = None
    for attempt in range(3):
        nc = _get_nc()
        old_m = nc.m
        nc.m = get_hw_module(nc.m)
        try:
            res = run_bass_kernel_spmd(nc, in_maps,
                                       core_ids=list(range(NCORES)))
        except Exception as e:           # wedged device etc: rebuild + retry
            last_err = e
            _NC_CACHE.clear()
            continue
        finally:
            nc.m = old_m
        loss = _post([res.results[c]["oej8"] for c in range(NCORES)],
                     [res.results[c]["oej16"] for c in range(NCORES)])
        if np.isfinite(loss):
            return loss
        last_err = ValueError(f"non-finite loss {loss}")
        _NC_CACHE.clear()
    raise last_err
